# revision 1
# baseline (speedup 1.0000x reference)
"""BlockAttentionResidual Trainium2 kernel.

Math (per (b,t) row, V slice v_n of length D, n = 0..7):
    ssq_n = sum(v_n^2)
    rms_n = rsqrt(ssq_n / D + eps)
    logit_n = rms_n * dot(v_n, qw)        with qw = key_norm_weight * pseudo_query
    w = softmax(logit)                     over n
    out = sum_n w_n * v_n

Sharding: rows (B*T flattened) split evenly across 8 cores; (D,) params
replicated. No cross-core communication.

Per-core layout: tiles of 32 rows; SBUF tile (128, 2D) float32r with
partition p = 8*r + n holding HBM-contiguous rows 2r and 2r+1 of plane n
(16 KiB per partition line -> efficient DMA; keeping n fastest across
partitions spreads concurrent SDMA streams over all planes, which
measured faster than plane-contiguous orderings).
  - ssq: ScalarE activation(Square, accum_out), one call per row-half
  - dot: VectorE scalar_tensor_tensor(mult, accum_out), one per row-half
  - rms = exp(-0.5*ln(ssq/D+eps)) on ScalarE (single ACT table set)
  - softmax over n: PE-transpose the (128, 2G) scalar columns so n lands
    innermost on the free axis, then max/exp/sum/div, transpose back
  - weighted sum: PE matmul in float32r (1 cyc/row vs fp32's 4);
    banded (128, 128) stationaries place row 32c+2r+eo at PSUM partition
    32c+2r+eo, 2G accumulating matmuls per 512-chunk, groups of 4 tiles
    (last 128 rows as single-tile groups to shrink the pipeline tail);
    PSUM->SBUF copy split ACT/DVE, full-width row-linear 1 MiB stores.
DMA rings: input tiles on the SP HWDGE ring, consts + output stores on
the ACT HWDGE ring.  Measured: 352.5 us on 8 cores, rel err 4.5e-4
(float32r rounding), DMA-bound at ~92% DMA occupancy.
"""

import os
import sys

for _p in ("/opt/trn_rl_repo",):
    if _p not in sys.path and os.path.isdir(_p):
        sys.path.append(_p)

import numpy as np

import concourse.bass as bass
import concourse.tile as tile
from concourse import bacc, mybir
from concourse.bass_utils import run_bass_kernel_spmd

N_CORES = 8
N = 8          # depth entries (softmax axis)
B = 4
T = 2048
D = 2048
R_TOTAL = B * T            # 8192 rows
RPC = R_TOTAL // N_CORES   # 1024 rows per core
TR = 32                    # rows per tile (16 r-slots x 2 rows x 8 n)
EPS = 1e-6
NCHUNK = 512               # matmul moving free-dim chunk (fp32 max)

F32 = mybir.dt.float32
BF16 = mybir.dt.bfloat16
F32R = mybir.dt.float32r
ALU = mybir.AluOpType
ACTF = mybir.ActivationFunctionType


def build_program(rows_per_core=RPC, debug=False, xbufs=9):
    """Build the per-core Bass program (identical on all cores)."""
    nt = rows_per_core // TR           # tiles per core
    nc = bacc.Bacc(
        "TRN2", target_bir_lowering=False, debug=debug, num_devices=N_CORES
    )

    v_dram = nc.dram_tensor("V", (N, rows_per_core, D), F32R, kind="ExternalInput").ap()
    qw_dram = nc.dram_tensor("QW", (128, D), F32, kind="ExternalInput").ap()
    em_dram = nc.dram_tensor("EM", (8, 128, 128), F32, kind="ExternalInput").ap()
    id_dram = nc.dram_tensor("ID", (128, 128), F32, kind="ExternalInput").ap()
    out_dram = nc.dram_tensor(
        "OUT", (rows_per_core, D), F32, kind="ExternalOutput"
    ).ap()

    with tile.TileContext(nc) as tc:
        with (
            tc.tile_pool(name="consts", bufs=1) as consts,
            tc.tile_pool(name="xpool", bufs=xbufs) as xpool,
            tc.tile_pool(name="scratch", bufs=1) as scratch,
            tc.tile_pool(name="outpool", bufs=2) as outpool,
            tc.tile_pool(name="smalls", bufs=3) as smalls,
            tc.tile_pool(name="wdpool", bufs=16) as wdpool,
            tc.tile_pool(name="psum_big", bufs=2, space="PSUM") as psum_big_pool,
            tc.tile_pool(name="psum_sm", bufs=2, space="PSUM") as psum_sm,
        ):
            qw_sb = consts.tile([128, D], F32)
            nc.scalar.dma_start(qw_sb[:], qw_dram[:])
            em_sb = []
            for i in range(8):
                em = consts.tile([128, 128], F32, tag=f"em{i}")
                nc.scalar.dma_start(em[:], em_dram[i])
                em_sb.append(em)
            id_sb = consts.tile([128, 128], F32)
            nc.scalar.dma_start(id_sb[:], id_dram[:])
            zero_sb = consts.tile([128, 1], F32)
            nc.vector.memset(zero_sb[:], 0.0)
            eps_sb = consts.tile([128, 1], F32)
            nc.vector.memset(eps_sb[:], EPS)

            assert nt % 4 == 0, "tiles per core must be a multiple of 4"

            def emit_group(tlist):
                G = len(tlist)
                xt = []
                dots = smalls.tile([128, 2 * G], F32, tag="dots")
                ssqs = smalls.tile([128, 2 * G], F32, tag="ssqs")
                for j, t in enumerate(tlist):
                    x = xpool.tile([128, 2 * D], F32R, tag="x")
                    # partition p = 8r+n <- rows (32t+2r, 32t+2r+1) of plane n
                    src = (
                        v_dram[:, TR * t : TR * (t + 1), :]
                        .rearrange("n (r two) d -> r n (two d)", two=2)
                    )
                    nc.sync.dma_start(x[:], src)
                    xt.append(x)

                    for eo in range(2):
                        xh = x[:, D * eo : D * (eo + 1)]
                        prod = scratch.tile([128, D], BF16, tag="prod")
                        nc.vector.scalar_tensor_tensor(
                            out=prod[:],
                            in0=xh.bitcast(F32),
                            scalar=1.0,
                            in1=qw_sb[:],
                            op0=ALU.mult,
                            op1=ALU.mult,
                            accum_out=dots[:, 2 * j + eo : 2 * j + eo + 1],
                        )
                        sq = scratch.tile([128, D], BF16, tag="sq")
                        nc.scalar.activation(
                            sq[:], xh.bitcast(F32), ACTF.Square, bias=zero_sb[:],
                            accum_out=ssqs[:, 2 * j + eo : 2 * j + eo + 1],
                        )

                # logits = dot * rsqrt(ssq/D + eps)  — (128, 8)
                lns = smalls.tile([128, 2 * G], F32, tag="lns")
                nc.scalar.activation(
                    lns[:], ssqs[:], ACTF.Ln, bias=eps_sb[:], scale=1.0 / D
                )
                rms = smalls.tile([128, 2 * G], F32, tag="rms")
                nc.scalar.activation(
                    rms[:], lns[:], ACTF.Exp, bias=zero_sb[:], scale=-0.5
                )
                logits = smalls.tile([128, 2 * G], F32, tag="logits")
                nc.vector.tensor_mul(logits[:], dots[:], rms[:])

                # transpose to (8, 128) so n is innermost on the free axis
                ps_t = psum_sm.tile([2 * G, 128], F32, tag="pst")
                nc.tensor.transpose(ps_t[:], logits[:], id_sb[:])
                tsb = smalls.tile([2 * G, 128], F32, tag="tsb")
                nc.scalar.copy(tsb[:], ps_t[:])
                t3 = tsb[:].rearrange("p (r n) -> p r n", n=N)

                negmax = smalls.tile([2 * G, 16], F32, tag="negmax")
                nc.vector.tensor_reduce(
                    negmax[:], t3, axis=mybir.AxisListType.X, op=ALU.max, negate=True
                )
                shifted = smalls.tile([2 * G, 128], F32, tag="shifted")
                sh3 = shifted[:].rearrange("p (r n) -> p r n", n=N)
                nmb = negmax[:].unsqueeze(2).broadcast_to([2 * G, 16, N])
                nc.vector.tensor_tensor(sh3, t3, nmb, ALU.add)
                expd = smalls.tile([2 * G, 128], F32, tag="expd")
                nc.scalar.activation(expd[:], shifted[:], ACTF.Exp, bias=zero_sb[0 : 2 * G])
                ex3 = expd[:].rearrange("p (r n) -> p r n", n=N)
                sums = smalls.tile([2 * G, 16], F32, tag="sums")
                nc.vector.tensor_reduce(
                    sums[:], ex3, axis=mybir.AxisListType.X, op=ALU.add
                )
                rsums = smalls.tile([2 * G, 16], F32, tag="rsums")
                nc.vector.reciprocal(rsums[:], sums[:])
                wts = smalls.tile([2 * G, 128], F32, tag="wts")
                w3 = wts[:].rearrange("p (r n) -> p r n", n=N)
                rsb = rsums[:].unsqueeze(2).broadcast_to([2 * G, 16, N])
                nc.vector.tensor_tensor(w3, ex3, rsb, ALU.mult)

                # transpose back: column 2j+eo = weights for (tile j, parity eo)
                ps_w = psum_sm.tile([128, 2 * G], F32, tag="psw")
                nc.tensor.transpose(ps_w[:], wts[:], id_sb[0 : 2 * G, 0 : 2 * G])
                wcols = smalls.tile([128, 2 * G], F32, tag="wcols")
                nc.scalar.copy(wcols[:], ps_w[:])

                # weighted sum via PE: banded (128,128) f32r stationaries,
                # 8 accumulating matmuls per 512-chunk (4 tiles x 2 parities)
                wds = []
                for c in range(G):
                    for eo in range(2):
                        wd = wdpool.tile([128, 128], F32R, tag="wd")
                        nc.vector.tensor_scalar(
                            out=wd[:], in0=em_sb[2 * c + eo][:],
                            scalar1=wcols[:, 2 * c + eo : 2 * c + eo + 1],
                            scalar2=None, op0=ALU.mult,
                        )
                        wds.append(wd)
                osb = outpool.tile([32 * G, D], F32, tag="osb")
                for h in range(2):
                    psb = psum_big_pool.tile([32 * G, D // 2], F32, tag="psb")
                    for kk in range(D // NCHUNK // 2):
                        k = h * (D // NCHUNK // 2) + kk
                        ps_slice = psb[:, NCHUNK * kk : NCHUNK * (kk + 1)]
                        for c in range(G):
                            for eo in range(2):
                                nc.tensor.matmul(
                                    ps_slice, wds[2 * c + eo][:, 0 : 32 * G],
                                    xt[c][:, D * eo + NCHUNK * k
                                           : D * eo + NCHUNK * (k + 1)],
                                    start=(c == 0 and eo == 0),
                                    stop=(c == G - 1 and eo == 1),
                                )
                    # split the PSUM->SBUF copy across ACT and DVE
                    eng = nc.scalar.copy if h == 0 else nc.vector.tensor_copy
                    eng(osb[:, h * (D // 2) : (h + 1) * (D // 2)], psb[:])
                # masks put row 32c+2r+eo at partition 32c+2r+eo: plain store
                r0 = TR * tlist[0]
                nc.scalar.dma_start(out_dram[r0 : r0 + 32 * G, :], osb[:])

            for g in range(nt // 4 - 1):
                emit_group([4 * g + j for j in range(4)])
            for t in range(nt - 4, nt):
                emit_group([t])

    nc.compile()
    return nc


def make_consts():
    """Host-side constants: even/odd block-diagonal masks and identity."""
    em = np.zeros((8, 128, 128), dtype=np.float32)
    for c in range(4):
        for eo in range(2):
            for p in range(128):
                r = p // N
                em[2 * c + eo, p, 32 * c + 2 * r + eo] = 1.0
    ident = np.eye(128, dtype=np.float32)
    return em, ident


def prepare_in_maps(V, key_norm_weight, pseudo_query, rows_per_core=RPC,
                    n_cores=N_CORES):
    qw = (np.asarray(key_norm_weight, dtype=np.float32)
          * np.asarray(pseudo_query, dtype=np.float32))
    qw_b = np.ascontiguousarray(np.broadcast_to(qw, (128, D)))
    em, ident = make_consts()
    vf = np.ascontiguousarray(np.asarray(V, dtype=np.float32)).reshape(N, -1, D)
    in_maps = []
    for c in range(n_cores):
        sl = np.ascontiguousarray(
            vf[:, c * rows_per_core : (c + 1) * rows_per_core, :]
        )
        in_maps.append({"V": sl, "QW": qw_b, "EM": em, "ID": ident})
    return in_maps


_PROGRAM_CACHE = {}


def _get_program():
    key = (RPC,)
    if key not in _PROGRAM_CACHE:
        _PROGRAM_CACHE[key] = build_program(RPC, debug=False)
    return _PROGRAM_CACHE[key]


def run(V, key_norm_weight, pseudo_query, trace=False, **trace_kwargs):
    nc = _get_program()
    in_maps = prepare_in_maps(V, key_norm_weight, pseudo_query)
    res = run_bass_kernel_spmd(
        nc, in_maps, list(range(N_CORES)), trace=trace, **trace_kwargs
    )
    out = np.empty((R_TOTAL, D), dtype=np.float32)
    for c in range(N_CORES):
        out[c * RPC : (c + 1) * RPC, :] = res.results[c]["OUT"]
    return out.reshape(B, T, D), res


def kernel(V, key_norm_weight, pseudo_query):
    out, _ = run(V, key_norm_weight, pseudo_query, trace=False)
    return out



# revision 5
# speedup vs baseline: 1.3104x; 1.3104x over previous
"""BlockAttentionResidual Trainium2 kernel (plane-major DMA layout).

Math (per (b,t) row, V slice v_n of length D, n = 0..7):
    ssq_n = sum(v_n^2)
    rms_n = rsqrt(ssq_n / D + eps)
    logit_n = rms_n * dot(v_n, qw)        with qw = key_norm_weight * pseudo_query
    w = softmax(logit)                     over n
    out = sum_n w_n * v_n

Sharding: rows (B*T flattened) split evenly across 8 cores; (D,) params
replicated. No cross-core communication.

Per-core layout: blocks of 256 rows. For each block, 8 plane tiles
[128, 2D] f32r with partition p holding HBM-contiguous rows (2p, 2p+1)
of one plane -> every load DMA is a fully contiguous 2 MiB HBM read
(16 KiB per partition line). Measured 387 GB/s vs 212 GB/s for the
plane-interleaved layout (HBM page locality, not descriptor size, is
what matters).
  - ssq: ScalarE activation(Square, accum_out) per row-half
  - dot: VectorE scalar_tensor_tensor(mult, accum_out) per row-half
  - rms = exp(-0.5*ln(ssq/D+eps)) on ScalarE
  - softmax over n: plane index is on the free axis ([128, 8] tiles),
    so max/exp/sum/div are direct vector ops - no transposes
  - weighted sum: PE matmul with per-plane diagonal f32r stationaries
    diag(w_eo[:, n]) built in one broadcast DVE op per parity;
    8 accumulating matmuls per 512-chunk per parity
  - output staged bf16 in SBUF (halves store traffic), host upcasts
DMA rings: loads on the sync HWDGE ring, consts + stores on the
scalar HWDGE ring.
"""

import os
import sys

for _p in ("/opt/trn_rl_repo",):
    if _p not in sys.path and os.path.isdir(_p):
        sys.path.append(_p)

import numpy as np

import concourse.bass as bass
import concourse.tile as tile
from concourse import bacc, mybir
from concourse.bass_utils import run_bass_kernel_spmd

N_CORES = 8
N = 8          # depth entries (softmax axis)
B = 4
T = 2048
D = 2048
R_TOTAL = B * T            # 8192 rows
RPC = R_TOTAL // N_CORES   # 1024 rows per core
BR = 256                   # rows per block (2 rows per partition)
EPS = 1e-6
NCHUNK = 512               # matmul moving free-dim chunk (fp32 max)

F32 = mybir.dt.float32
BF16 = mybir.dt.bfloat16
F32R = mybir.dt.float32r
ALU = mybir.AluOpType
ACTF = mybir.ActivationFunctionType


def build_program(rows_per_core=RPC, debug=False, xbufs=10):
    """Build the per-core Bass program (identical on all cores)."""
    nb = rows_per_core // BR           # blocks per core
    nc = bacc.Bacc(
        "TRN2", target_bir_lowering=False, debug=debug, num_devices=N_CORES
    )

    v_dram = nc.dram_tensor("V", (N, rows_per_core, D), F32R, kind="ExternalInput").ap()
    qw_dram = nc.dram_tensor("QW", (128, D), F32, kind="ExternalInput").ap()
    id_dram = nc.dram_tensor("ID", (128, 128), F32, kind="ExternalInput").ap()
    out_dram = nc.dram_tensor(
        "OUT", (rows_per_core, D), BF16, kind="ExternalOutput"
    ).ap()

    with tile.TileContext(nc) as tc:
        with (
            tc.tile_pool(name="consts", bufs=1) as consts,
            tc.tile_pool(name="xpool", bufs=xbufs) as xpool,
            tc.tile_pool(name="scratch", bufs=1) as scratch,
            tc.tile_pool(name="outpool", bufs=1) as outpool,
            tc.tile_pool(name="smalls", bufs=3) as smalls,
            tc.tile_pool(name="wdpool", bufs=2) as wdpool,
            tc.tile_pool(name="psum_big", bufs=2, space="PSUM") as psum_big_pool,
        ):
            qw_sb = consts.tile([128, D], F32)
            nc.scalar.dma_start(qw_sb[:], qw_dram[:])
            id_sb = consts.tile([128, 128], F32)
            nc.scalar.dma_start(id_sb[:], id_dram[:])
            zero_sb = consts.tile([128, 1], F32)
            nc.vector.memset(zero_sb[:], 0.0)
            eps_sb = consts.tile([128, 1], F32)
            nc.vector.memset(eps_sb[:], EPS)

            def emit_block(b):
                xt = []
                dots0 = smalls.tile([128, N], F32, tag="dots0")
                dots1 = smalls.tile([128, N], F32, tag="dots1")
                ssqs0 = smalls.tile([128, N], F32, tag="ssqs0")
                ssqs1 = smalls.tile([128, N], F32, tag="ssqs1")
                dots = [dots0, dots1]
                ssqs = [ssqs0, ssqs1]
                for n in range(N):
                    x = xpool.tile([128, 2 * D], F32R, tag="x")
                    # partition p <- rows (BR*b + 2p, BR*b + 2p + 1) of plane n
                    src = (
                        v_dram[n, BR * b : BR * (b + 1), :]
                        .rearrange("(p two) d -> p (two d)", two=2)
                    )
                    nc.sync.dma_start(x[:], src)
                    xt.append(x)

                    for eo in range(2):
                        xh = x[:, D * eo : D * (eo + 1)]
                        prod = scratch.tile([128, D], BF16, tag="prod")
                        nc.vector.scalar_tensor_tensor(
                            out=prod[:],
                            in0=xh.bitcast(F32),
                            scalar=1.0,
                            in1=qw_sb[:],
                            op0=ALU.mult,
                            op1=ALU.mult,
                            accum_out=dots[eo][:, n : n + 1],
                        )
                        sq = scratch.tile([128, D], BF16, tag="sq")
                        nc.scalar.activation(
                            sq[:], xh.bitcast(F32), ACTF.Square, bias=zero_sb[:],
                            accum_out=ssqs[eo][:, n : n + 1],
                        )

                # softmax over n (free axis) per parity -> diag stationaries
                wds = []
                for eo in range(2):
                    lns = smalls.tile([128, N], F32, tag=f"lns{eo}")
                    nc.scalar.activation(
                        lns[:], ssqs[eo][:], ACTF.Ln, bias=eps_sb[:], scale=1.0 / D
                    )
                    rms = smalls.tile([128, N], F32, tag=f"rms{eo}")
                    nc.scalar.activation(
                        rms[:], lns[:], ACTF.Exp, bias=zero_sb[:], scale=-0.5
                    )
                    logits = smalls.tile([128, N], F32, tag=f"logits{eo}")
                    nc.vector.tensor_mul(logits[:], dots[eo][:], rms[:])
                    negmax = smalls.tile([128, 1], F32, tag=f"negmax{eo}")
                    nc.vector.tensor_reduce(
                        negmax[:], logits[:], axis=mybir.AxisListType.X,
                        op=ALU.max, negate=True,
                    )
                    shifted = smalls.tile([128, N], F32, tag=f"shifted{eo}")
                    nc.vector.tensor_tensor(
                        shifted[:], logits[:], negmax[:].broadcast_to([128, N]),
                        ALU.add,
                    )
                    expd = smalls.tile([128, N], F32, tag=f"expd{eo}")
                    nc.scalar.activation(expd[:], shifted[:], ACTF.Exp, bias=zero_sb[:])
                    sums = smalls.tile([128, 1], F32, tag=f"sums{eo}")
                    nc.vector.tensor_reduce(
                        sums[:], expd[:], axis=mybir.AxisListType.X, op=ALU.add
                    )
                    rsums = smalls.tile([128, 1], F32, tag=f"rsums{eo}")
                    nc.vector.reciprocal(rsums[:], sums[:])
                    wts = smalls.tile([128, N], F32, tag=f"wts{eo}")
                    nc.vector.tensor_tensor(
                        wts[:], expd[:], rsums[:].broadcast_to([128, N]), ALU.mult
                    )
                    # wd[:, 128n+m] = wts[:, n] * id[:, m] -> diag(w[:, n]) blocks
                    wd = wdpool.tile([128, N * 128], F32R, tag=f"wd{eo}")
                    wd3 = wd[:].rearrange("p (n m) -> p n m", m=128)
                    nc.vector.tensor_tensor(
                        wd3,
                        wts[:].unsqueeze(2).broadcast_to([128, N, 128]),
                        id_sb[:].unsqueeze(1).broadcast_to([128, N, 128]),
                        ALU.mult,
                    )
                    wds.append(wd)

                # weighted sum: per (parity, D-half), 8 accumulating diag
                # matmuls per 512-chunk
                osb = outpool.tile([128, 2 * D], BF16, tag="osb")
                for eo in range(2):
                    for h in range(2):
                        psb = psum_big_pool.tile([128, D // 2], F32, tag="psb")
                        for n in range(N):
                            lhsT = wds[eo][:, 128 * n : 128 * (n + 1)]
                            for kk in range(D // NCHUNK // 2):
                                k = h * (D // NCHUNK // 2) + kk
                                nc.tensor.matmul(
                                    psb[:, NCHUNK * kk : NCHUNK * (kk + 1)],
                                    lhsT,
                                    xt[n][:, D * eo + NCHUNK * k
                                           : D * eo + NCHUNK * (k + 1)],
                                    start=(n == 0),
                                    stop=(n == N - 1),
                                )
                        # split the PSUM->SBUF copies across ACT and DVE
                        eng = nc.scalar.copy if h == 0 else nc.vector.tensor_copy
                        eng(
                            osb[:, D * eo + h * (D // 2)
                                   : D * eo + (h + 1) * (D // 2)],
                            psb[:],
                        )
                # partition p holds rows (2p, 2p+1): contiguous 1 MiB store
                dst = (
                    out_dram[BR * b : BR * (b + 1), :]
                    .rearrange("(p two) d -> p (two d)", two=2)
                )
                nc.scalar.dma_start(dst, osb[:])

            for b in range(nb):
                emit_block(b)

    nc.compile()
    return nc


def prepare_in_maps(V, key_norm_weight, pseudo_query, rows_per_core=RPC,
                    n_cores=N_CORES):
    qw = (np.asarray(key_norm_weight, dtype=np.float32)
          * np.asarray(pseudo_query, dtype=np.float32))
    qw_b = np.ascontiguousarray(np.broadcast_to(qw, (128, D)))
    ident = np.eye(128, dtype=np.float32)
    vf = np.ascontiguousarray(np.asarray(V, dtype=np.float32)).reshape(N, -1, D)
    in_maps = []
    for c in range(n_cores):
        sl = np.ascontiguousarray(
            vf[:, c * rows_per_core : (c + 1) * rows_per_core, :]
        )
        in_maps.append({"V": sl, "QW": qw_b, "ID": ident})
    return in_maps


_PROGRAM_CACHE = {}


def _get_program():
    key = (RPC,)
    if key not in _PROGRAM_CACHE:
        _PROGRAM_CACHE[key] = build_program(RPC, debug=False)
    return _PROGRAM_CACHE[key]


def run(V, key_norm_weight, pseudo_query, trace=False, **trace_kwargs):
    nc = _get_program()
    in_maps = prepare_in_maps(V, key_norm_weight, pseudo_query)
    res = run_bass_kernel_spmd(
        nc, in_maps, list(range(N_CORES)), trace=trace, **trace_kwargs
    )
    out = np.empty((R_TOTAL, D), dtype=np.float32)
    for c in range(N_CORES):
        out[c * RPC : (c + 1) * RPC, :] = np.asarray(
            res.results[c]["OUT"]
        ).astype(np.float32)
    return out.reshape(B, T, D), res


def kernel(V, key_norm_weight, pseudo_query):
    out, _ = run(V, key_norm_weight, pseudo_query, trace=False)
    return out


# revision 11
# speedup vs baseline: 1.5588x; 1.1895x over previous
"""BlockAttentionResidual Trainium2 kernel (plane-major DMA layout).

Math (per (b,t) row, V slice v_n of length D, n = 0..7):
    ssq_n = sum(v_n^2)
    rms_n = rsqrt(ssq_n / D + eps)
    logit_n = rms_n * dot(v_n, qw)        with qw = key_norm_weight * pseudo_query
    w = softmax(logit)                     over n
    out = sum_n w_n * v_n

Sharding: rows (B*T flattened) split evenly across 8 cores; (D,) params
replicated. No cross-core communication.

Per-core layout: blocks of 256 rows. For each block, 8 plane tiles
[128, 2D] f32r with partition p holding HBM-contiguous rows (2p, 2p+1)
of one plane -> every load DMA is a fully contiguous 2 MiB HBM read
(16 KiB per partition line). Measured 387 GB/s vs 212 GB/s for the
plane-interleaved layout (HBM page locality, not descriptor size, is
what matters).
  - ssq: ScalarE activation(Square, accum_out) per row-half
  - dot: VectorE scalar_tensor_tensor(mult, accum_out) per row-half
  - rms = exp(-0.5*ln(ssq/D+eps)) on ScalarE
  - softmax over n: plane index is on the free axis ([128, 8] tiles),
    so max/exp/sum/div are direct vector ops - no transposes
  - weighted sum: PE matmul with per-plane diagonal f32r stationaries
    diag(w_eo[:, n]) built in one broadcast DVE op per parity;
    8 accumulating matmuls per 512-chunk per parity
  - output staged bf16 in SBUF (halves store traffic), host upcasts
DMA rings: loads on the sync HWDGE ring, consts + stores on the
scalar HWDGE ring.
"""

import os
import sys

for _p in ("/opt/trn_rl_repo",):
    if _p not in sys.path and os.path.isdir(_p):
        sys.path.append(_p)

import numpy as np

import concourse.bass as bass
import concourse.tile as tile
from concourse import bacc, mybir
from concourse.bass_utils import run_bass_kernel_spmd

N_CORES = 8
N = 8          # depth entries (softmax axis)
B = 4
T = 2048
D = 2048
R_TOTAL = B * T            # 8192 rows
RPC = R_TOTAL // N_CORES   # 1024 rows per core
BR = 256                   # rows per block (2 rows per partition)
EPS = 1e-6
NCHUNK = 512               # matmul moving free-dim chunk (fp32 max)

F32 = mybir.dt.float32
BF16 = mybir.dt.bfloat16
F32R = mybir.dt.float32r
ALU = mybir.AluOpType
ACTF = mybir.ActivationFunctionType


def build_program(rows_per_core=RPC, debug=False, xbufs=10):
    """Build the per-core Bass program (identical on all cores)."""
    nb = rows_per_core // BR           # blocks per core
    nc = bacc.Bacc(
        "TRN2", target_bir_lowering=False, debug=debug, num_devices=N_CORES
    )

    v_dram = nc.dram_tensor("V", (N, rows_per_core, D), F32R, kind="ExternalInput").ap()
    qw_dram = nc.dram_tensor("QW", (128, D), F32, kind="ExternalInput").ap()
    id_dram = nc.dram_tensor("ID", (128, 128), F32, kind="ExternalInput").ap()
    out_dram = nc.dram_tensor(
        "OUT", (rows_per_core, D), BF16, kind="ExternalOutput"
    ).ap()

    with tile.TileContext(nc) as tc:
        with (
            tc.tile_pool(name="consts", bufs=1) as consts,
            tc.tile_pool(name="xpool", bufs=xbufs) as xpool,
            tc.tile_pool(name="scratch", bufs=1) as scratch,
            tc.tile_pool(name="outpool", bufs=1) as outpool,
            tc.tile_pool(name="smalls", bufs=3) as smalls,
            tc.tile_pool(name="wdpool", bufs=2) as wdpool,
            tc.tile_pool(name="psum_big", bufs=2, space="PSUM") as psum_big_pool,
        ):
            qw_sb = consts.tile([128, D], F32)
            nc.scalar.dma_start(qw_sb[:], qw_dram[:])
            id_sb = consts.tile([128, 128], F32)
            nc.scalar.dma_start(id_sb[:], id_dram[:])
            zero_sb = consts.tile([128, 1], F32)
            nc.vector.memset(zero_sb[:], 0.0)
            eps_sb = consts.tile([128, 1], F32)
            nc.vector.memset(eps_sb[:], EPS)

            def emit_block(b):
                xt = []
                dots0 = smalls.tile([128, N], F32, tag="dots0")
                dots1 = smalls.tile([128, N], F32, tag="dots1")
                ssqs0 = smalls.tile([128, N], F32, tag="ssqs0")
                ssqs1 = smalls.tile([128, N], F32, tag="ssqs1")
                dots = [dots0, dots1]
                ssqs = [ssqs0, ssqs1]
                for n in range(N):
                    x = xpool.tile([128, 2 * D], F32R, tag="x")
                    # partition p <- rows (BR*b + 2p, BR*b + 2p + 1) of plane n
                    src = (
                        v_dram[n, BR * b : BR * (b + 1), :]
                        .rearrange("(p two) d -> p (two d)", two=2)
                    )
                    nc.sync.dma_start(x[:], src)
                    xt.append(x)

                    for eo in range(2):
                        xh = x[:, D * eo : D * (eo + 1)]
                        prod = scratch.tile([128, D], BF16, tag="prod")
                        nc.vector.scalar_tensor_tensor(
                            out=prod[:],
                            in0=xh.bitcast(F32),
                            scalar=1.0,
                            in1=qw_sb[:],
                            op0=ALU.mult,
                            op1=ALU.mult,
                            accum_out=dots[eo][:, n : n + 1],
                        )
                        sq = scratch.tile([128, D], BF16, tag="sq")
                        nc.scalar.activation(
                            sq[:], xh.bitcast(F32), ACTF.Square, bias=zero_sb[:],
                            accum_out=ssqs[eo][:, n : n + 1],
                        )

                # softmax over n (free axis) per parity -> diag stationaries
                wds = []
                for eo in range(2):
                    lns = smalls.tile([128, N], F32, tag=f"lns{eo}")
                    nc.scalar.activation(
                        lns[:], ssqs[eo][:], ACTF.Ln, bias=eps_sb[:], scale=1.0 / D
                    )
                    rms = smalls.tile([128, N], F32, tag=f"rms{eo}")
                    nc.scalar.activation(
                        rms[:], lns[:], ACTF.Exp, bias=zero_sb[:], scale=-0.5
                    )
                    logits = smalls.tile([128, N], F32, tag=f"logits{eo}")
                    nc.vector.tensor_mul(logits[:], dots[eo][:], rms[:])
                    negmax = smalls.tile([128, 1], F32, tag=f"negmax{eo}")
                    nc.vector.tensor_reduce(
                        negmax[:], logits[:], axis=mybir.AxisListType.X,
                        op=ALU.max, negate=True,
                    )
                    shifted = smalls.tile([128, N], F32, tag=f"shifted{eo}")
                    nc.vector.tensor_tensor(
                        shifted[:], logits[:], negmax[:].broadcast_to([128, N]),
                        ALU.add,
                    )
                    expd = smalls.tile([128, N], F32, tag=f"expd{eo}")
                    nc.scalar.activation(expd[:], shifted[:], ACTF.Exp, bias=zero_sb[:])
                    sums = smalls.tile([128, 1], F32, tag=f"sums{eo}")
                    nc.vector.tensor_reduce(
                        sums[:], expd[:], axis=mybir.AxisListType.X, op=ALU.add
                    )
                    rsums = smalls.tile([128, 1], F32, tag=f"rsums{eo}")
                    nc.vector.reciprocal(rsums[:], sums[:])
                    wts = smalls.tile([128, N], F32, tag=f"wts{eo}")
                    nc.vector.tensor_tensor(
                        wts[:], expd[:], rsums[:].broadcast_to([128, N]), ALU.mult
                    )
                    # wd[:, 128n+m] = wts[:, n] * id[:, m] -> diag(w[:, n])
                    wd = wdpool.tile([128, N * 128], F32R, tag=f"wd{eo}")
                    for n in range(N):
                        nc.vector.tensor_scalar(
                            out=wd[:, 128 * n : 128 * (n + 1)],
                            in0=id_sb[:],
                            scalar1=wts[:, n : n + 1],
                            scalar2=None,
                            op0=ALU.mult,
                        )
                    wds.append(wd)

                # weighted sum: per (parity, D-half), 8 accumulating diag
                # matmuls per 512-chunk
                osb = outpool.tile([128, 2 * D], BF16, tag="osb")
                for eo in range(2):
                    for h in range(2):
                        psb = psum_big_pool.tile([128, D // 2], F32, tag="psb")
                        for n in range(N):
                            lhsT = wds[eo][:, 128 * n : 128 * (n + 1)]
                            for kk in range(D // NCHUNK // 2):
                                k = h * (D // NCHUNK // 2) + kk
                                nc.tensor.matmul(
                                    psb[:, NCHUNK * kk : NCHUNK * (kk + 1)],
                                    lhsT,
                                    xt[n][:, D * eo + NCHUNK * k
                                           : D * eo + NCHUNK * (k + 1)],
                                    start=(n == 0),
                                    stop=(n == N - 1),
                                )
                        # split the PSUM->SBUF copies across ACT and DVE
                        eng = nc.scalar.copy if h == 0 else nc.vector.tensor_copy
                        eng(
                            osb[:, D * eo + h * (D // 2)
                                   : D * eo + (h + 1) * (D // 2)],
                            psb[:],
                        )
                # partition p holds rows (2p, 2p+1): contiguous 1 MiB store
                dst = (
                    out_dram[BR * b : BR * (b + 1), :]
                    .rearrange("(p two) d -> p (two d)", two=2)
                )
                nc.scalar.dma_start(dst, osb[:])

            for b in range(nb):
                emit_block(b)

    nc.compile()
    return nc


def prepare_in_maps(V, key_norm_weight, pseudo_query, rows_per_core=RPC,
                    n_cores=N_CORES):
    qw = (np.asarray(key_norm_weight, dtype=np.float32)
          * np.asarray(pseudo_query, dtype=np.float32))
    qw_b = np.ascontiguousarray(np.broadcast_to(qw, (128, D)))
    ident = np.eye(128, dtype=np.float32)
    vf = np.ascontiguousarray(np.asarray(V, dtype=np.float32)).reshape(N, -1, D)
    in_maps = []
    for c in range(n_cores):
        sl = np.ascontiguousarray(
            vf[:, c * rows_per_core : (c + 1) * rows_per_core, :]
        )
        in_maps.append({"V": sl, "QW": qw_b, "ID": ident})
    return in_maps


_PROGRAM_CACHE = {}


def _get_program():
    key = (RPC,)
    if key not in _PROGRAM_CACHE:
        _PROGRAM_CACHE[key] = build_program(RPC, debug=False)
    return _PROGRAM_CACHE[key]


def run(V, key_norm_weight, pseudo_query, trace=False, **trace_kwargs):
    nc = _get_program()
    in_maps = prepare_in_maps(V, key_norm_weight, pseudo_query)
    res = run_bass_kernel_spmd(
        nc, in_maps, list(range(N_CORES)), trace=trace, **trace_kwargs
    )
    out = np.empty((R_TOTAL, D), dtype=np.float32)
    for c in range(N_CORES):
        out[c * RPC : (c + 1) * RPC, :] = np.asarray(
            res.results[c]["OUT"]
        ).astype(np.float32)
    return out.reshape(B, T, D), res


def kernel(V, key_norm_weight, pseudo_query):
    out, _ = run(V, key_norm_weight, pseudo_query, trace=False)
    return out


# revision 12
# speedup vs baseline: 1.9888x; 1.2758x over previous
"""BlockAttentionResidual Trainium2 kernel (plane-major fp16 pipeline).

Math (per (b,t) row, V slice v_n of length D, n = 0..7):
    ssq_n = sum(v_n^2)
    rms_n = rsqrt(ssq_n / D + eps)
    logit_n = rms_n * dot(v_n, qw)        with qw = key_norm_weight * pseudo_query
    w = softmax(logit)                     over n
    out = sum_n w_n * v_n

Sharding: rows (B*T flattened) split evenly across 8 cores; (D,) params
replicated. No cross-core communication.

Per-core layout: blocks of 256 rows. For each block, 8 plane tiles
[128, 2D] with partition p holding HBM-contiguous rows (2p, 2p+1) of one
plane -> every load is a fully contiguous 2 MiB HBM read. Loads go
through SWDGE (gpsimd) with an inline f32 -> fp16 cast: HBM reads stay
f32 (required bytes) but every downstream engine pass runs at 16-bit
rates (DVE 2x mode, ScalarE 2x, half-size LDWEIGHTS) and SBUF tiles
halve, buying a 2.5-block prefetch depth.
  - ssq: ScalarE activation(Square, accum_out) per row-half
  - dot: VectorE scalar_tensor_tensor(mult, accum_out) per row-half
  - rms = exp(-0.5*ln(ssq/D+eps)) on ScalarE
  - softmax over n: plane index is on the free axis ([128, 8] tiles),
    direct vector ops, no transposes
  - weighted sum: PE matmul, fp16 diag stationaries diag(w_eo[:, n])
    built by per-plane tensor_scalar; 8 accumulating matmuls per
    512-chunk per (parity, D-half)
  - output staged bf16 in SBUF (halves store traffic), host upcasts
Precision (numpy-simulated): fp16 x/qw dot noise sigma~0.03 on logits,
fp16 weights, bf16 store -> rel err ~2.3e-3 (gate 2e-2).
DMA rings: loads on SWDGE, consts + stores on the scalar HWDGE ring.
"""

import os
import sys

for _p in ("/opt/trn_rl_repo",):
    if _p not in sys.path and os.path.isdir(_p):
        sys.path.append(_p)

import numpy as np

import concourse.bass as bass
import concourse.tile as tile
from concourse import bacc, mybir
from concourse.bass_utils import run_bass_kernel_spmd

N_CORES = 8
N = 8          # depth entries (softmax axis)
B = 4
T = 2048
D = 2048
R_TOTAL = B * T            # 8192 rows
RPC = R_TOTAL // N_CORES   # 1024 rows per core
BR = 256                   # rows per block (2 rows per partition)
EPS = 1e-6
NCHUNK = 512               # matmul moving free-dim chunk

F32 = mybir.dt.float32
BF16 = mybir.dt.bfloat16
F16 = mybir.dt.float16
ALU = mybir.AluOpType
ACTF = mybir.ActivationFunctionType


def build_program(rows_per_core=RPC, debug=False, xbufs=20):
    """Build the per-core Bass program (identical on all cores)."""
    nb = rows_per_core // BR           # blocks per core
    nc = bacc.Bacc(
        "TRN2", target_bir_lowering=False, debug=debug, num_devices=N_CORES
    )

    v_dram = nc.dram_tensor("V", (N, rows_per_core, D), F32, kind="ExternalInput").ap()
    qw_dram = nc.dram_tensor("QW", (128, D), F16, kind="ExternalInput").ap()
    id_dram = nc.dram_tensor("ID", (128, 128), F16, kind="ExternalInput").ap()
    out_dram = nc.dram_tensor(
        "OUT", (rows_per_core, D), BF16, kind="ExternalOutput"
    ).ap()

    with tile.TileContext(nc) as tc:
        with (
            tc.tile_pool(name="consts", bufs=1) as consts,
            tc.tile_pool(name="xpool", bufs=xbufs) as xpool,
            tc.tile_pool(name="scratch", bufs=1) as scratch,
            tc.tile_pool(name="outpool", bufs=2) as outpool,
            tc.tile_pool(name="smalls", bufs=3) as smalls,
            tc.tile_pool(name="wdpool", bufs=2) as wdpool,
            tc.tile_pool(name="psum_big", bufs=2, space="PSUM") as psum_big_pool,
        ):
            qw_sb = consts.tile([128, D], F16)
            nc.scalar.dma_start(qw_sb[:], qw_dram[:])
            id_sb = consts.tile([128, 128], F16)
            nc.scalar.dma_start(id_sb[:], id_dram[:])
            zero_sb = consts.tile([128, 1], F32)
            nc.vector.memset(zero_sb[:], 0.0)
            eps_sb = consts.tile([128, 1], F32)
            nc.vector.memset(eps_sb[:], EPS)

            def emit_block(b):
                xt = []
                dots0 = smalls.tile([128, N], F32, tag="dots0")
                dots1 = smalls.tile([128, N], F32, tag="dots1")
                ssqs0 = smalls.tile([128, N], F32, tag="ssqs0")
                ssqs1 = smalls.tile([128, N], F32, tag="ssqs1")
                dots = [dots0, dots1]
                ssqs = [ssqs0, ssqs1]
                for n in range(N):
                    x = xpool.tile([128, 2 * D], F16, tag="x")
                    # partition p <- rows (BR*b + 2p, BR*b + 2p + 1) of plane n
                    src = (
                        v_dram[n, BR * b : BR * (b + 1), :]
                        .rearrange("(p two) d -> p (two d)", two=2)
                    )
                    nc.gpsimd.dma_start(x[:], src)
                    xt.append(x)

                    for eo in range(2):
                        xh = x[:, D * eo : D * (eo + 1)]
                        prod = scratch.tile([128, D], F16, tag="prod")
                        nc.vector.scalar_tensor_tensor(
                            out=prod[:],
                            in0=xh,
                            scalar=1.0,
                            in1=qw_sb[:],
                            op0=ALU.mult,
                            op1=ALU.mult,
                            accum_out=dots[eo][:, n : n + 1],
                        )
                        sq = scratch.tile([128, D], F16, tag="sq")
                        nc.scalar.activation(
                            sq[:], xh, ACTF.Square, bias=zero_sb[:],
                            accum_out=ssqs[eo][:, n : n + 1],
                        )

                # softmax over n (free axis) per parity -> diag stationaries
                wds = []
                for eo in range(2):
                    lns = smalls.tile([128, N], F32, tag=f"lns{eo}")
                    nc.scalar.activation(
                        lns[:], ssqs[eo][:], ACTF.Ln, bias=eps_sb[:], scale=1.0 / D
                    )
                    rms = smalls.tile([128, N], F32, tag=f"rms{eo}")
                    nc.scalar.activation(
                        rms[:], lns[:], ACTF.Exp, bias=zero_sb[:], scale=-0.5
                    )
                    logits = smalls.tile([128, N], F32, tag=f"logits{eo}")
                    nc.vector.tensor_mul(logits[:], dots[eo][:], rms[:])
                    negmax = smalls.tile([128, 1], F32, tag=f"negmax{eo}")
                    nc.vector.tensor_reduce(
                        negmax[:], logits[:], axis=mybir.AxisListType.X,
                        op=ALU.max, negate=True,
                    )
                    shifted = smalls.tile([128, N], F32, tag=f"shifted{eo}")
                    nc.vector.tensor_tensor(
                        shifted[:], logits[:], negmax[:].broadcast_to([128, N]),
                        ALU.add,
                    )
                    expd = smalls.tile([128, N], F32, tag=f"expd{eo}")
                    nc.scalar.activation(expd[:], shifted[:], ACTF.Exp, bias=zero_sb[:])
                    sums = smalls.tile([128, 1], F32, tag=f"sums{eo}")
                    nc.vector.tensor_reduce(
                        sums[:], expd[:], axis=mybir.AxisListType.X, op=ALU.add
                    )
                    rsums = smalls.tile([128, 1], F32, tag=f"rsums{eo}")
                    nc.vector.reciprocal(rsums[:], sums[:])
                    wts = smalls.tile([128, N], F32, tag=f"wts{eo}")
                    nc.vector.tensor_tensor(
                        wts[:], expd[:], rsums[:].broadcast_to([128, N]), ALU.mult
                    )
                    # wd[:, 128n+m] = wts[:, n] * id[:, m] -> diag(w[:, n])
                    wd = wdpool.tile([128, N * 128], F16, tag=f"wd{eo}")
                    for n in range(N):
                        nc.vector.tensor_scalar(
                            out=wd[:, 128 * n : 128 * (n + 1)],
                            in0=id_sb[:],
                            scalar1=wts[:, n : n + 1],
                            scalar2=None,
                            op0=ALU.mult,
                        )
                    wds.append(wd)

                # weighted sum: per (parity, D-half), 8 accumulating diag
                # matmuls per 512-chunk
                osb = outpool.tile([128, 2 * D], BF16, tag="osb")
                for eo in range(2):
                    for h in range(2):
                        psb = psum_big_pool.tile([128, D // 2], F32, tag="psb")
                        for n in range(N):
                            lhsT = wds[eo][:, 128 * n : 128 * (n + 1)]
                            for kk in range(D // NCHUNK // 2):
                                k = h * (D // NCHUNK // 2) + kk
                                nc.tensor.matmul(
                                    psb[:, NCHUNK * kk : NCHUNK * (kk + 1)],
                                    lhsT,
                                    xt[n][:, D * eo + NCHUNK * k
                                           : D * eo + NCHUNK * (k + 1)],
                                    start=(n == 0),
                                    stop=(n == N - 1),
                                )
                        # split the PSUM->SBUF copies across ACT and DVE
                        eng = nc.scalar.copy if h == 0 else nc.vector.tensor_copy
                        eng(
                            osb[:, D * eo + h * (D // 2)
                                   : D * eo + (h + 1) * (D // 2)],
                            psb[:],
                        )
                # partition p holds rows (2p, 2p+1): contiguous 1 MiB store
                dst = (
                    out_dram[BR * b : BR * (b + 1), :]
                    .rearrange("(p two) d -> p (two d)", two=2)
                )
                nc.scalar.dma_start(dst, osb[:])

            for b in range(nb):
                emit_block(b)

    nc.compile()
    return nc


def prepare_in_maps(V, key_norm_weight, pseudo_query, rows_per_core=RPC,
                    n_cores=N_CORES):
    qw = (np.asarray(key_norm_weight, dtype=np.float32)
          * np.asarray(pseudo_query, dtype=np.float32)).astype(np.float16)
    qw_b = np.ascontiguousarray(np.broadcast_to(qw, (128, D)))
    ident = np.eye(128, dtype=np.float16)
    vf = np.ascontiguousarray(np.asarray(V, dtype=np.float32)).reshape(N, -1, D)
    in_maps = []
    for c in range(n_cores):
        sl = np.ascontiguousarray(
            vf[:, c * rows_per_core : (c + 1) * rows_per_core, :]
        )
        in_maps.append({"V": sl, "QW": qw_b, "ID": ident})
    return in_maps


_PROGRAM_CACHE = {}


def _get_program():
    key = (RPC,)
    if key not in _PROGRAM_CACHE:
        _PROGRAM_CACHE[key] = build_program(RPC, debug=False)
    return _PROGRAM_CACHE[key]


def run(V, key_norm_weight, pseudo_query, trace=False, **trace_kwargs):
    nc = _get_program()
    in_maps = prepare_in_maps(V, key_norm_weight, pseudo_query)
    res = run_bass_kernel_spmd(
        nc, in_maps, list(range(N_CORES)), trace=trace, **trace_kwargs
    )
    out = np.empty((R_TOTAL, D), dtype=np.float32)
    for c in range(N_CORES):
        out[c * RPC : (c + 1) * RPC, :] = np.asarray(
            res.results[c]["OUT"]
        ).astype(np.float32)
    return out.reshape(B, T, D), res


def kernel(V, key_norm_weight, pseudo_query):
    out, _ = run(V, key_norm_weight, pseudo_query, trace=False)
    return out
